# revision 17
# baseline (speedup 1.0000x reference)
"""Trainium2 Bass kernel for a 24-layer Qwen2-style decoder with a custom
(token-type dependent) attention mask.

Sharding: sequence-parallel. 8 cores = (batch b in {0,1}) x (4 quarters of
the 2048-token sequence). Each core owns T=512 query tokens end-to-end
(norms, QKV, attention over the full 2048 keys, MLP, residual). The only
cross-core communication is a per-layer AllGather of the RoPE'd K and V
(bf16, 256KB -> 1MB) within the 4-core group that shares a batch element.

On-device layout is feature-major ("transposed activations"): activations
live as [feature, token] so every matmul's output directly feeds the next
matmul's moving operand.

Key scheduling idea (v2): RMS-norm scaling is DEFERRED past the
projections.  q_unnorm = wq^T x is computed straight from the (bf16 shadow
of the) residual stream, and the per-token 1/rms factor is folded into the
RoPE cos/sin tables (attention) or applied to the PSUM result (v / MLP).
The TensorEngine therefore never waits on the norm reduction chain, which
keeps it dense enough to hold the 2.4 GHz activity clock.  The
sum-of-squares reduction for the *next* norm is interleaved into the
residual-add tail of the previous block.

Weights are packed host-side so each DMA moves contiguous per-partition
lines (1.8-9.7 KB), whole-layer wq/wo in one descriptor-efficient DMA.

The mask is precomputed on the host as a multiplicative {0,1} mask applied
to exp(scores) (exact: exp(s + min_float) == 0 == exp(s) * 0).
"""

import sys

for _p in ("/opt/trn_rl_repo",):
    if _p not in sys.path:
        sys.path.insert(0, _p)

import numpy as np
import ml_dtypes

import concourse.bass as bass
import concourse.mybir as mybir
import concourse.tile as tile
from concourse import bacc
from concourse.bass_utils import run_bass_kernel_spmd
from concourse.masks import make_identity

F32 = mybir.dt.float32
BF16 = mybir.dt.bfloat16
AF = mybir.ActivationFunctionType

# model dims
D = 896
L = 24
HQ = 14
HKV = 2
DH = 64
I = 4864
EPS = 1e-6
THETA = 1e6
B = 2
S = 2048

# sharding
N_CORES = 8
G = 4                      # cores per batch group
T = S // G                 # 512 local query tokens per core
GROUPS = [[0, 1, 2, 3], [4, 5, 6, 7]]

KC = D // 128              # 7   K-chunks over hidden dim
MI = I // 128              # 38  M-chunks over intermediate dim
NKC = S // 128             # 16  chunks over key dim
MCH = 3                    # wg/wu DMA chunk (mi granularity)
WDH = MI // 2              # 19  wd DMA half-chunk

N_LAYERS_OVERRIDE = None   # for testing with fewer layers

_BUILD_CACHE = {}
_LAST_IN_MAPS = None
DEBUG_DUMP = False


def _build(n_layers, with_bias):
    assert not with_bias, "bias path not implemented (biases are zero)"
    nc = bacc.Bacc(num_devices=N_CORES)

    xT_p = nc.declare_dram_parameter("xT", [D, T], F32, isOutput=False)
    cos_p = nc.declare_dram_parameter("cosb", [128, T], F32, isOutput=False)
    sin_p = nc.declare_dram_parameter("sinb", [128, T], F32, isOutput=False)
    mask_p = nc.declare_dram_parameter("maskT", [128, NKC, T], BF16, isOutput=False)
    rotm_p = nc.declare_dram_parameter("rotm", [128, 128], BF16, isOutput=False)
    wq_p = nc.declare_dram_parameter("wq", [n_layers, 128, KC, KC, 128], BF16, isOutput=False)
    wk_p = nc.declare_dram_parameter("wk", [n_layers, 128, KC, 128], BF16, isOutput=False)
    wv_p = nc.declare_dram_parameter("wv", [n_layers, 128, KC, 128], BF16, isOutput=False)
    wo_p = nc.declare_dram_parameter("wo", [n_layers, 128, KC, KC, 128], BF16, isOutput=False)
    wg_p = nc.declare_dram_parameter("wg", [n_layers, 128, MI, KC, 128], BF16, isOutput=False)
    wu_p = nc.declare_dram_parameter("wu", [n_layers, 128, MI, KC, 128], BF16, isOutput=False)
    wd_p = nc.declare_dram_parameter("wd", [n_layers, 128, KC, MI, 128], BF16, isOutput=False)
    outT_p = nc.declare_dram_parameter("outT", [D, T], F32, isOutput=True)
    if DEBUG_DUMP:
        dbg_q = nc.declare_dram_parameter("dbg_q", [128, KC, T], F32, isOutput=True)
        dbg_k = nc.declare_dram_parameter("dbg_k", [128, T], F32, isOutput=True)
        dbg_o = nc.declare_dram_parameter("dbg_o", [128, KC, T], F32, isOutput=True)
        dbg_x = nc.declare_dram_parameter("dbg_x", [128, KC, T], F32, isOutput=True)
        dbg_r = nc.declare_dram_parameter("dbg_r", [128, T], F32, isOutput=True)

    with tile.TileContext(nc) as tc:
        with tc.tile_pool(name="const", bufs=1) as constp, \
             tc.tile_pool(name="persist", bufs=1) as persist, \
             tc.tile_pool(name="wqp", bufs=1) as wqp, \
             tc.tile_pool(name="wop", bufs=1) as wop, \
             tc.tile_pool(name="wkvp", bufs=2) as wkvp, \
             tc.tile_pool(name="wgup", bufs=2) as wgup, \
             tc.tile_pool(name="wdp", bufs=2) as wdp, \
             tc.tile_pool(name="attn", bufs=1) as attnp, \
             tc.tile_pool(name="mpool", bufs=1) as mpool, \
             tc.tile_pool(name="norm", bufs=1) as normp, \
             tc.tile_pool(name="small", bufs=2) as smallp, \
             tc.tile_pool(name="tmp16", bufs=4) as tmp16p, \
             tc.tile_pool(name="expp", bufs=3) as expp, \
             tc.tile_pool(name="ps", bufs=1, space="PSUM") as ps, \
             tc.tile_pool(name="dramp", bufs=2, space="DRAM") as dramp:

            ident = constp.tile([128, 128], BF16)
            make_identity(nc, ident)
            ones_col = constp.tile([128, 1], BF16)
            nc.vector.memset(ones_col, 1.0)
            eps_t = constp.tile([1, 1], F32)
            nc.vector.memset(eps_t, EPS)
            rotm_sb = constp.tile([128, 128], BF16)
            nc.sync.dma_start(out=rotm_sb, in_=rotm_p[:, :])
            cos_sb = constp.tile([128, T], F32)
            nc.sync.dma_start(out=cos_sb, in_=cos_p[:, :])
            sin_sb = constp.tile([128, T], F32)
            nc.sync.dma_start(out=sin_sb, in_=sin_p[:, :])
            mask_sb = constp.tile([128, NKC, T], BF16)
            nc.sync.dma_start(out=mask_sb, in_=mask_p[:, :, :])

            xT_sb = persist.tile([128, KC, T], F32)
            nc.sync.dma_start(out=xT_sb, in_=xT_p.rearrange("(kc p) t -> p kc t", p=128))
            xb_sb = persist.tile([128, KC, T], BF16)
            nc.scalar.copy(xb_sb, xT_sb)

            # initial ssq for layer-0 rstd1
            ssq = ps.tile([1, T], F32, tag="ssq", bufs=1)
            for k in range(KC):
                sq = tmp16p.tile([128, T], BF16, tag="t16")
                nc.vector.tensor_mul(sq, xT_sb[:, k, :], xT_sb[:, k, :])
                nc.tensor.matmul(ssq, ones_col, sq, start=(k == 0), stop=(k == KC - 1))

            def rstd_chain(ssq_ps, tag):
                """ssq(PSUM) -> rstd broadcast [128,T] f32 in SBUF."""
                rmsv = smallp.tile([1, T], F32, tag="rmsv")
                nc.scalar.activation(out=rmsv, in_=ssq_ps, func=AF.Sqrt,
                                     bias=eps_t, scale=1.0 / D)
                rstd = smallp.tile([1, T], BF16, tag="rstd")
                with nc.allow_low_precision(reason="bf16 rstd broadcast"):
                    nc.vector.reciprocal(rstd, rmsv)
                rb = normp.tile([128, T], BF16, tag=tag)
                nc.gpsimd.partition_broadcast(rb, rstd[:1, :])
                return rb

            for l in range(n_layers):
                # ---- weight prefetch (Tile schedules as early as slots allow)
                wk_sb = wkvp.tile([128, KC, 128], BF16, tag="wk")
                nc.sync.dma_start(out=wk_sb, in_=wk_p[l])
                wv_sb = wkvp.tile([128, KC, 128], BF16, tag="wv")
                nc.sync.dma_start(out=wv_sb, in_=wv_p[l])
                wq_sb = wqp.tile([128, KC, KC, 128], BF16, tag="wq")
                nc.sync.dma_start(out=wq_sb, in_=wq_p[l])
                wo_sb = wop.tile([128, KC, KC, 128], BF16, tag="wo")
                nc.sync.dma_start(out=wo_sb, in_=wo_p[l])

                # ---- rstd1 chain (concurrent with k/v/q projections below)
                rstd1b = rstd_chain(ssq, "rstd1b")
                cosr = normp.tile([128, T], BF16, tag="cosr")
                nc.vector.tensor_mul(cosr, cos_sb, rstd1b)
                sinr = normp.tile([128, T], BF16, tag="sinr")
                nc.vector.tensor_mul(sinr, sin_sb, rstd1b)

                def proj_rope(wa_sb, out_ap):
                    """out = (rope(wa^T x)) * rstd, feature-major."""
                    pa = ps.tile([128, T], F32, tag="w", bufs=3)
                    for k in range(KC):
                        nc.tensor.matmul(pa, wa_sb[:, k, :], xb_sb[:, k, :],
                                         start=(k == 0), stop=(k == KC - 1))
                    qa_sb = tmp16p.tile([128, T], BF16, tag="t16")
                    nc.scalar.copy(qa_sb, pa)
                    pb = ps.tile([128, T], F32, tag="w", bufs=3)
                    nc.tensor.matmul(pb, rotm_sb, qa_sb, start=True, stop=True)
                    ta = tmp16p.tile([128, T], BF16, tag="t16")
                    nc.vector.tensor_mul(ta, pa, cosr)
                    tb = tmp16p.tile([128, T], BF16, tag="t16")
                    nc.vector.tensor_mul(tb, pb, sinr)
                    nc.vector.tensor_add(out_ap, ta, tb)

                # local k (RoPE'd) and v first so the allgather starts early
                kT_loc = attnp.tile([128, T], BF16, tag="kT_loc")
                proj_rope(wk_sb, kT_loc[:, :])
                pv = ps.tile([128, T], F32, tag="w", bufs=3)
                for k in range(KC):
                    nc.tensor.matmul(pv, wv_sb[:, k, :], xb_sb[:, k, :],
                                     start=(k == 0), stop=(k == KC - 1))
                vT_loc = attnp.tile([128, T], BF16, tag="vT_loc")
                nc.vector.tensor_mul(vT_loc, pv, rstd1b)

                kv_in = dramp.tile([2, 128, T], BF16, tag="kv_in")
                nc.sync.dma_start(out=kv_in[0], in_=kT_loc[:, :])
                nc.sync.dma_start(out=kv_in[1], in_=vT_loc[:, :])
                kv_out = dramp.tile([G, 2, 128, T], BF16, tag="kv_out")
                nc.gpsimd.collective_compute(
                    "AllGather", mybir.AluOpType.bypass,
                    replica_groups=GROUPS,
                    ins=[kv_in.opt()], outs=[kv_out.opt()],
                )

                # q projections (chunk mc holds heads mc, mc+7)
                qT = attnp.tile([128, KC, T], BF16, tag="qT")
                for mc in range(KC):
                    proj_rope(wq_sb[:, mc], qT[:, mc, :])

                kT_full = attnp.tile([128, S], BF16, tag="kT_full")
                vT_full = attnp.tile([128, S], BF16, tag="vT_full")
                for r in range(G):
                    nc.sync.dma_start(out=kT_full[:, r * T:(r + 1) * T], in_=kv_out[r, 0])
                    nc.sync.dma_start(out=vT_full[:, r * T:(r + 1) * T], in_=kv_out[r, 1])

                # V natural layout + ones column: [v | ones] -> AV output rows
                # 0..63 = o, row 64 = softmax denominator
                vaug = attnp.tile([128, NKC, HKV, 65], BF16, tag="vaug")
                nc.vector.memset(vaug[:, :, :, 64:65], 1.0)
                for kc in range(NKC):
                    tr = ps.tile([128, 128], BF16, tag="w", bufs=3)
                    nc.tensor.transpose(tr, vT_full[:, kc * 128:(kc + 1) * 128], ident)
                    for g in range(HKV):
                        nc.vector.tensor_copy(vaug[:, kc, g, 0:64], tr[:, g * 64:g * 64 + 64])

                # attention, head pairs (mc, mc+7). Head h sits at partition
                # base (h//7)*64 == its kv-group's base in kT_full.
                oT = attnp.tile([128, KC, T], BF16, tag="oT")
                for mc in range(KC):
                    for half in range(2):
                        g = half
                        oacc = ps.tile([65, T], F32, tag="oacc", bufs=2)
                        for kc in range(NKC):
                            sT = ps.tile([128, T], F32, tag="sT", bufs=2)
                            nc.tensor.matmul(
                                sT,
                                kT_full[g * 64:g * 64 + 64, kc * 128:(kc + 1) * 128],
                                qT[half * 64:half * 64 + 64, mc, :],
                                start=True, stop=True)
                            ex = expp.tile([128, T], BF16, tag="expT")
                            nc.scalar.activation(out=ex, in_=sT, func=AF.Exp, scale=0.125)
                            nc.vector.tensor_mul(ex, ex, mask_sb[:, kc, :])
                            nc.tensor.matmul(oacc[0:65, :], vaug[:, kc, g, :], ex,
                                             start=(kc == 0), stop=(kc == NKC - 1))
                        den = smallp.tile([65, T], F32, tag="den")
                        nc.scalar.copy(den[64:65, :], oacc[64:65, :])
                        den0 = smallp.tile([1, T], F32, tag="den0")
                        nc.gpsimd.tensor_copy(out=den0, in_=den[64:65, :])
                        rec0 = smallp.tile([1, T], BF16, tag="rec0")
                        with nc.allow_low_precision(reason="bf16 softmax recip"):
                            nc.vector.reciprocal(rec0, den0)
                        recb = smallp.tile([64, T], BF16, tag="recb")
                        nc.gpsimd.partition_broadcast(recb, rec0[:1, :])
                        if half == 0:
                            nc.vector.tensor_mul(oT[0:64, mc, :], oacc[0:64, :], recb)
                        else:
                            oTs = smallp.tile([64, T], BF16, tag="oTs")
                            nc.vector.tensor_mul(oTs, oacc[0:64, :], recb)
                            nc.gpsimd.tensor_copy(out=oT[64:128, mc, :], in_=oTs)

                if DEBUG_DUMP and l == 0:
                    for mc in range(KC):
                        dq = smallp.tile([128, T], F32, tag="outc")
                        nc.vector.tensor_copy(dq, qT[:, mc, :])
                        nc.sync.dma_start(out=dbg_q[:, mc, :], in_=dq)
                        do_ = smallp.tile([128, T], F32, tag="outc")
                        nc.vector.tensor_copy(do_, oT[:, mc, :])
                        nc.sync.dma_start(out=dbg_o[:, mc, :], in_=do_)
                    dk = smallp.tile([128, T], F32, tag="outc")
                    nc.vector.tensor_copy(dk, kT_loc)
                    nc.sync.dma_start(out=dbg_k[:, :], in_=dk)
                    dr = smallp.tile([128, T], F32, tag="outc")
                    nc.vector.tensor_copy(dr, rstd1b)
                    nc.sync.dma_start(out=dbg_r[:, :], in_=dr)

                # o_proj + residual + next-norm ssq accumulation
                ssq = ps.tile([1, T], F32, tag="ssq", bufs=1)
                for mc in range(KC):
                    xd = ps.tile([128, T], F32, tag="w", bufs=3)
                    for k in range(KC):
                        nc.tensor.matmul(xd, wo_sb[:, mc, k, :], oT[:, k, :],
                                         start=(k == 0), stop=(k == KC - 1))
                    nc.vector.tensor_add(xT_sb[:, mc, :], xT_sb[:, mc, :], xd)
                    nc.scalar.copy(xb_sb[:, mc, :], xT_sb[:, mc, :])
                    sq = tmp16p.tile([128, T], BF16, tag="t16")
                    nc.vector.tensor_mul(sq, xT_sb[:, mc, :], xT_sb[:, mc, :])
                    nc.tensor.matmul(ssq, ones_col, sq, start=(mc == 0), stop=(mc == KC - 1))

                if DEBUG_DUMP and l == 0:
                    for mc in range(KC):
                        dx = smallp.tile([128, T], F32, tag="outc")
                        nc.vector.tensor_copy(dx, xT_sb[:, mc, :])
                        nc.sync.dma_start(out=dbg_x[:, mc, :], in_=dx)

                # ---------------- MLP ----------------
                rstd2b = rstd_chain(ssq, "rstd2b")
                m_sb = mpool.tile([128, MI, T], BF16, tag="m")
                mi0 = 0
                while mi0 < MI:
                    ch = min(MCH, MI - mi0)
                    wg_sb = wgup.tile([128, MCH, KC, 128], BF16, tag="wg")
                    nc.sync.dma_start(out=wg_sb[:, :ch], in_=wg_p[l, :, mi0:mi0 + ch])
                    wu_sb = wgup.tile([128, MCH, KC, 128], BF16, tag="wu")
                    nc.sync.dma_start(out=wu_sb[:, :ch], in_=wu_p[l, :, mi0:mi0 + ch])
                    for j in range(ch):
                        mi = mi0 + j
                        pg = ps.tile([128, T], F32, tag="w", bufs=3)
                        for k in range(KC):
                            nc.tensor.matmul(pg, wg_sb[:, j, k, :], xb_sb[:, k, :],
                                             start=(k == 0), stop=(k == KC - 1))
                        pu = ps.tile([128, T], F32, tag="w", bufs=3)
                        for k in range(KC):
                            nc.tensor.matmul(pu, wu_sb[:, j, k, :], xb_sb[:, k, :],
                                             start=(k == 0), stop=(k == KC - 1))
                        t1 = tmp16p.tile([128, T], BF16, tag="t16")
                        nc.vector.tensor_mul(t1, pg, rstd2b)
                        sg = tmp16p.tile([128, T], BF16, tag="t16")
                        nc.scalar.activation(out=sg, in_=t1, func=AF.Silu)
                        t2 = tmp16p.tile([128, T], BF16, tag="t16")
                        nc.vector.tensor_mul(t2, pu, rstd2b)
                        nc.vector.tensor_mul(m_sb[:, mi, :], sg, t2)
                    mi0 += ch

                # down proj + residual + next-layer ssq
                ssq = ps.tile([1, T], F32, tag="ssq", bufs=1)
                for mc in range(KC):
                    xd = ps.tile([128, T], F32, tag="w", bufs=3)
                    for sub in range(2):
                        wd_sb = wdp.tile([128, WDH, 128], BF16, tag="wd")
                        nc.sync.dma_start(out=wd_sb,
                                          in_=wd_p[l, :, mc, sub * WDH:(sub + 1) * WDH])
                        for j in range(WDH):
                            ki = sub * WDH + j
                            nc.tensor.matmul(xd, wd_sb[:, j, :], m_sb[:, ki, :],
                                             start=(ki == 0), stop=(ki == MI - 1))
                    nc.vector.tensor_add(xT_sb[:, mc, :], xT_sb[:, mc, :], xd)
                    nc.scalar.copy(xb_sb[:, mc, :], xT_sb[:, mc, :])
                    sq = tmp16p.tile([128, T], BF16, tag="t16")
                    nc.vector.tensor_mul(sq, xT_sb[:, mc, :], xT_sb[:, mc, :])
                    nc.tensor.matmul(ssq, ones_col, sq, start=(mc == 0), stop=(mc == KC - 1))

            # final norm (lnf applied on host)
            rstdfb = rstd_chain(ssq, "rstd1b")
            outT_r = outT_p.rearrange("(kc p) t -> p kc t", p=128)
            for k in range(KC):
                oc = smallp.tile([128, T], F32, tag="outc")
                nc.vector.tensor_mul(oc, xT_sb[:, k, :], rstdfb)
                nc.sync.dma_start(out=outT_r[:, k, :], in_=oc)

    nc.finalize()
    return nc


def get_kernel(n_layers, with_bias):
    key = (n_layers, with_bias)
    if key not in _BUILD_CACHE:
        _BUILD_CACHE[key] = _build(n_layers, with_bias)
    return _BUILD_CACHE[key]


def _bf(a):
    return np.asarray(a, dtype=np.float32).astype(ml_dtypes.bfloat16)


def _pack4(w):
    """[Din, Dout] -> [p, mc, kc, m]: contiguous per-partition DMA lines."""
    din, dout = w.shape
    return np.ascontiguousarray(
        w.reshape(din // 128, 128, dout // 128, 128).transpose(1, 2, 0, 3))


def _pack3(w):
    """[Din, 128] -> [p, kc, m]."""
    din = w.shape[0]
    return np.ascontiguousarray(w.reshape(din // 128, 128, 128).transpose(1, 0, 2))


def prepare_in_maps(inputs, n_layers, with_bias):
    return _prepare(n_layers=n_layers, with_bias_override=with_bias, **inputs)[0]


def _prepare(inputs_embeds, token_type_ids, attention_mask,
             wq, bq, wk, bk, wv, bv, wo, wg, wu, wd, ln1, ln2, lnf,
             n_layers=None, with_bias_override=None):
    f32 = np.float32
    inputs_embeds = np.asarray(inputs_embeds, f32)
    token_type_ids = np.asarray(token_type_ids)
    attention_mask = np.asarray(attention_mask, f32)
    wq, bq, wk, bk = np.asarray(wq, f32), np.asarray(bq, f32), np.asarray(wk, f32), np.asarray(bk, f32)
    wv, bv, wo = np.asarray(wv, f32), np.asarray(bv, f32), np.asarray(wo, f32)
    wg, wu, wd = np.asarray(wg, f32), np.asarray(wu, f32), np.asarray(wd, f32)
    ln1, ln2, lnf = np.asarray(ln1, f32), np.asarray(ln2, f32), np.asarray(lnf, f32)

    if n_layers is None:
        n_layers = N_LAYERS_OVERRIDE if N_LAYERS_OVERRIDE is not None else L
    with_bias = bool(np.any(bq[:n_layers]) or np.any(bk[:n_layers]) or np.any(bv[:n_layers]))
    if with_bias_override is not None:
        with_bias = with_bias or with_bias_override
    assert not with_bias, "bias path not implemented (reference biases are zero)"

    # head permutation: q-chunk mc holds heads (mc, mc+7) so that each head's
    # partition half matches its GQA kv-group's rows in kT_full
    perm = [h for p in range(KC) for h in (p, p + KC)]
    inv_sl = np.array(perm)

    def _perm_qcols(w):                    # permute 64-wide head column blocks
        return np.ascontiguousarray(
            w.reshape(w.shape[0], HQ, DH)[:, inv_sl, :].reshape(w.shape[0], HQ * DH))

    def _perm_orows(w):                    # permute 64-wide head row blocks
        return np.ascontiguousarray(
            w.reshape(HQ, DH, w.shape[1])[inv_sl].reshape(HQ * DH, w.shape[1]))

    wq_eff = ln1[:, :, None] * wq          # [L, D, 896]
    wk_eff = ln1[:, :, None] * wk          # [L, D, 128]
    wv_eff = ln1[:, :, None] * wv
    wg_eff = ln2[:, :, None] * wg
    wu_eff = ln2[:, :, None] * wu

    # _pack4: wq/wo -> [p, mc, kc, m]; wg/wu -> [p, mi, kc, m]; wd -> [p, mc, ki, m]
    wq_pack = np.stack([_pack4(_perm_qcols(wq_eff[l])) for l in range(n_layers)])
    wo_pack = np.stack([_pack4(_perm_orows(wo[l])) for l in range(n_layers)])
    wg_pack = np.stack([_pack4(wg_eff[l]) for l in range(n_layers)])
    wu_pack = np.stack([_pack4(wu_eff[l]) for l in range(n_layers)])
    wd_pack = np.stack([_pack4(wd[l]) for l in range(n_layers)])
    wk_pack = np.stack([_pack3(wk_eff[l]) for l in range(n_layers)])
    wv_pack = np.stack([_pack3(wv_eff[l]) for l in range(n_layers)])

    # block-diag rotate-half matrix (two 64-head blocks), as lhsT = R^T
    r64 = np.zeros((DH, DH), np.float32)
    r64[:DH // 2, DH // 2:] = -np.eye(DH // 2, dtype=np.float32)
    r64[DH // 2:, :DH // 2] = np.eye(DH // 2, dtype=np.float32)
    rot2 = np.zeros((128, 128), np.float32)
    rot2[:DH, :DH] = r64.T
    rot2[DH:, DH:] = r64.T

    base = {
        "wq": _bf(wq_pack), "wk": _bf(wk_pack), "wv": _bf(wv_pack),
        "wo": _bf(wo_pack), "wg": _bf(wg_pack), "wu": _bf(wu_pack), "wd": _bf(wd_pack),
        "rotm": _bf(rot2),
    }

    # ---- RoPE tables ----
    inv_freq = 1.0 / (THETA ** (np.arange(0, DH, 2, dtype=f32) / DH))
    ang = np.arange(S, dtype=f32)[:, None] * inv_freq[None, :]      # [S, 32]
    emb = np.concatenate([ang, ang], axis=-1)                        # [S, DH]
    cos_full, sin_full = np.cos(emb), np.sin(emb)                    # [S, DH]

    # ---- mask (multiplicative, per batch) ----
    t = token_type_ids
    tq = t[:, :, None]
    tk = t[:, None, :]
    qi = np.arange(S)[:, None]
    ki = np.arange(S)[None, :]
    allowed = ((tq == 0) & (tk == 0)) | ((tq == 1) & ((tk == 0) | ((tk == 1) & (ki <= qi))))
    m = allowed.astype(f32) * (attention_mask[:, None, :] > 0.5)     # [B, S(q), S(k)]

    in_maps = []
    for c in range(N_CORES):
        b, qt = c // G, c % G
        q0 = qt * T
        im = dict(base)
        im["xT"] = np.ascontiguousarray(inputs_embeds[b, q0:q0 + T, :].T)
        im["cosb"] = np.ascontiguousarray(np.tile(cos_full[q0:q0 + T].T, (2, 1)).astype(f32))
        im["sinb"] = np.ascontiguousarray(np.tile(sin_full[q0:q0 + T].T, (2, 1)).astype(f32))
        mT = np.ascontiguousarray(m[b, q0:q0 + T, :].T)              # [S(k), T(q)]
        im["maskT"] = _bf(np.ascontiguousarray(
            mT.reshape(NKC, 128, T).transpose(1, 0, 2)))
        in_maps.append(im)

    global _LAST_IN_MAPS
    _LAST_IN_MAPS = in_maps
    return in_maps, n_layers, with_bias


def kernel(**inputs):
    in_maps, n_layers, with_bias = _prepare(**inputs)
    nc = get_kernel(n_layers, with_bias)
    res = run_bass_kernel_spmd(nc, in_maps, list(range(N_CORES)))
    lnf = np.asarray(inputs["lnf"], np.float32)
    out = np.empty((B, S, D), dtype=np.float32)
    for c in range(N_CORES):
        b, qt = c // G, c % G
        out[b, qt * T:(qt + 1) * T, :] = res.results[c]["outT"].T
    out *= lnf[None, None, :]
    return out
